# revision 30
# baseline (speedup 1.0000x reference)
"""Trainium2 Bass kernel for nn_AttentionBlock (GroupNorm -> 1x1 qkv conv ->
softmax attention over N=HW -> 1x1 proj -> residual).

Sharding: 8 cores = 4 images x 2 query-column halves. Each core receives its
image column-permuted so its own 2048 query columns come first; attention is
permutation-invariant over key/value positions, so k/v use all 4096 columns
in permuted order. GroupNorm stats are computed on-chip per core (full image).

Math folding done on host (tiny O(C^2) numpy):
  - gn_w folded into qkv weight columns; gn_b folded into qkv biases.
  - 1/sqrt(C) score scale folded into Wq and bq.
  - k bias dropped entirely (adds a per-row constant to scores: softmax-invariant).
  - v bias folded into proj bias (softmax rows sum to 1): bp_eff = bp + Wp @ bv.
On-chip per core:
  h = (x - mean_g) * rstd_g              (per-channel affine from group stats)
  q = Wq^T-matmul(h) + bq  (cols 0:2048) ; k = Wk-matmul(h) (all cols)
  vpos[m, c] = Wv-matmul(h)              (position-major layout)
  per 512-col tile of q:  E[m, n] = exp(k_chunk^T q_tile) accumulated flash-style:
     av[c, n] += vpos_chunk^T E ;  eacc[n] += E (DVE)
  S = ones^T eacc (all-ones 128x128 matmul -> S replicated on all partitions)
  ha = av * (1/S) ; y = x_tile + Wp-matmul(ha) + bp_eff
"""

import numpy as np

B, C, HH, WW = 4, 256, 64, 64
N = HH * WW            # 4096
NH = N // 2            # 2048 query columns per core
GROUPS = 32
GSIZE = C // GROUPS    # 8
EPS = 1e-5
NCORES = 8
P = 128
NT = NH // 512         # 4 query tiles per core
MC = N // P            # 32 key chunks
KT = N // 512          # 8 column tiles for k

_prog = None


def _build_program():
    import concourse.bacc as bacc
    import concourse.tile as tile
    from concourse import mybir

    f32 = mybir.dt.float32
    f32r = mybir.dt.float32r
    AF = mybir.ActivationFunctionType
    ALU = mybir.AluOpType

    nc = bacc.Bacc("TRN2", target_bir_lowering=False, debug=False,
                   num_devices=NCORES)

    x_d = nc.dram_tensor("x", [C, N], f32r, kind="ExternalInput").ap()
    wqk_d = nc.dram_tensor("wqk", [C, 2 * C], f32r, kind="ExternalInput").ap()
    wv_d = nc.dram_tensor("wv", [C, C], f32r, kind="ExternalInput").ap()
    wp_d = nc.dram_tensor("wp", [C, C], f32r, kind="ExternalInput").ap()
    bq_d = nc.dram_tensor("bq", [C, 1], f32, kind="ExternalInput").ap()
    bp_d = nc.dram_tensor("bp", [C, 1], f32, kind="ExternalInput").ap()
    gm_d = nc.dram_tensor("gm", [P, 16], f32, kind="ExternalInput").ap()
    gt_d = nc.dram_tensor("gt", [16, P], f32, kind="ExternalInput").ap()
    onr_d = nc.dram_tensor("onr", [P, P], f32r, kind="ExternalInput").ap()
    y_d = nc.dram_tensor("y", [C, NH], f32, kind="ExternalOutput").ap()

    xv = x_d.rearrange("(j p) n -> p j n", p=P)        # [128, 2, 4096]
    wqkv = wqk_d.rearrange("(j p) o -> p j o", p=P)    # [128, 2, 512]
    wvv = wv_d.rearrange("(j p) o -> p j o", p=P)
    wpv = wp_d.rearrange("(j p) o -> p j o", p=P)
    bqv = bq_d.rearrange("(j p) o -> p j o", p=P)      # [128, 2, 1]
    bpv = bp_d.rearrange("(j p) o -> p j o", p=P)
    yv = y_d.rearrange("(j p) n -> p j n", p=P)        # [128, 2, 2048]

    with tile.TileContext(nc) as tc:
        with (
            tc.tile_pool(name="big", bufs=1) as big,
            tc.tile_pool(name="wts", bufs=1) as wts,
            tc.tile_pool(name="stats", bufs=1) as stats,
            tc.tile_pool(name="epool", bufs=6) as epool,
            tc.tile_pool(name="acc", bufs=2) as accp,
            tc.tile_pool(name="rp", bufs=2) as rp,
            tc.tile_pool(name="hap", bufs=2) as hap,
            tc.tile_pool(name="yp", bufs=2) as yp,
        ):

            # PE warmup: dense dummy matmuls fill the x-DMA wait so the HAM
            # clock gate opens (K=8/8) before the real matmul stream starts.
            dummy = wts.tile([P, 512], f32)
            nc.vector.memset(dummy, 0.0)
            with tc.tile_pool(name="psW", bufs=1, space="PSUM") as psw:
                wps = psw.tile([P, 512], f32, tag="w")
                dr = dummy.bitcast(f32r)
                for _ in range(82):
                    nc.tensor.matmul(wps, lhsT=dr[:, 0:P], rhs=dr,
                                     start=True, stop=True)

            # ---- load x first (critical path), 3 parallel DMA queues ----
            xs = big.tile([P, 2, N], f32r)
            dma_engs = [nc.sync, nc.gpsimd, nc.scalar, nc.sync]
            for j in range(2):
                for qd in range(4):
                    sl = slice(qd * 1024, (qd + 1) * 1024)
                    dma_engs[(j * 4 + qd) % 3].dma_start(
                        out=xs[:, j, sl], in_=xv[:, j, sl])

            # ---- weights / consts (off the critical path) ----
            wqk = wts.tile([P, 2, 2 * C], f32r)
            nc.gpsimd.dma_start(out=wqk, in_=wqkv)
            wv = wts.tile([P, 2, C], f32r)
            nc.scalar.dma_start(out=wv, in_=wvv)
            wp = wts.tile([P, 2, C], f32r)
            nc.scalar.dma_start(out=wp, in_=wpv)
            bq = wts.tile([P, 2, 1], f32)
            nc.sync.dma_start(out=bq, in_=bqv)
            bp = wts.tile([P, 2, 1], f32)
            nc.sync.dma_start(out=bp, in_=bpv)
            gm = wts.tile([P, 16], f32)
            nc.sync.dma_start(out=gm, in_=gm_d)
            gt = wts.tile([16, P], f32)
            nc.sync.dma_start(out=gt, in_=gt_d)
            ones_sq = wts.tile([P, P], f32r)
            nc.sync.dma_start(out=ones_sq, in_=onr_d)
            eps_t = wts.tile([16, 1], f32)
            nc.vector.memset(eps_t, EPS)

            # ---- group stats ----
            AB = stats.tile([P, 2, 2], f32)  # per-channel (mean, rstd)
            with tc.tile_pool(name="psStat", bufs=1, space="PSUM") as psst:
                for j in range(2):
                    st6 = stats.tile([P, 8, 6], f32, tag="st6")
                    xsr = xs.bitcast(f32)[:, j, :].rearrange("p (s f) -> p s f", f=512)
                    for sg in range(8):
                        nc.vector.bn_stats(out=st6[:, sg, :], in_=xsr[:, sg, :])
                    mv = stats.tile([P, 2], f32, tag="mv")
                    nc.vector.bn_aggr(out=mv, in_=st6)
                    # t2 = (mean, var + mean^2)
                    t2 = stats.tile([P, 2], f32, tag="t2")
                    nc.vector.tensor_copy(out=t2[:, 0:1], in_=mv[:, 0:1])
                    nc.vector.scalar_tensor_tensor(
                        out=t2[:, 1:2], in0=mv[:, 0:1], scalar=mv[:, 0:1],
                        in1=mv[:, 1:2], op0=ALU.mult, op1=ALU.add,
                    )
                    gagg = psst.tile([16, 2], f32, tag="gagg")
                    nc.tensor.matmul(gagg, lhsT=gm, rhs=t2, start=True, stop=True)
                    # grs = (gmean, rstd)
                    grs = stats.tile([16, 2], f32, tag="grs")
                    nc.scalar.copy(out=grs[:, 0:1], in_=gagg[:, 0:1])
                    sq = stats.tile([16, 1], f32, tag="sq")
                    nc.scalar.square(out=sq, in_=gagg[:, 0:1])
                    var = stats.tile([16, 1], f32, tag="var")
                    nc.vector.tensor_sub(out=var, in0=gagg[:, 1:2], in1=sq)
                    nc.scalar.activation(out=var, in_=var, func=AF.Sqrt,
                                         bias=eps_t, scale=1.0)
                    nc.vector.reciprocal(out=grs[:, 1:2], in_=var)
                    gb = psst.tile([P, 2], f32, tag="gb")
                    nc.tensor.matmul(gb, lhsT=gt, rhs=grs, start=True, stop=True)
                    nc.scalar.copy(out=AB[:, j, :], in_=gb)

            # bridge the PE clock gate through the normalize (DVE) phase
            with tc.tile_pool(name="psW2", bufs=1, space="PSUM") as psw2:
                wps2 = psw2.tile([P, 512], f32, tag="w2")
                dr2 = dummy.bitcast(f32r)
                for _ in range(25):
                    nc.tensor.matmul(wps2, lhsT=dr2[:, 0:P], rhs=dr2,
                                     start=True, stop=True)

            # ---- qkv: fold GroupNorm into weights, read raw x ----
            # W'' = W' * rstd[c_in] (per-partition row scale, ~1.5us on 1.5K
            # cols instead of normalizing 32K cols of x); mu correction goes
            # into the biases via tiny matvecs:
            #   q: bqn = bq - Wq''@mu ; k: dropped (softmax-invariant)
            #   v: y gets -Wp@(Wv''@mu) folded into the proj bias
            q_s = big.tile([P, 2, NH], f32r)
            k_s = big.tile([P, 2, N], f32r)
            v_s = big.tile([P, MC, C], f32r)
            with tc.tile_pool(name="psD", bufs=4, space="PSUM") as psd:
                wqk_e = wts.tile([P, 2, 2 * C], f32r)
                wv_e = wts.tile([P, 2, C], f32r)
                for j in range(2):
                    nc.vector.tensor_scalar_mul(
                        out=wqk_e[:, j, :], in0=wqk.bitcast(f32)[:, j, :],
                        scalar1=AB[:, j, 1:2])
                    nc.vector.tensor_scalar_mul(
                        out=wv_e[:, j, :], in0=wv.bitcast(f32)[:, j, :],
                        scalar1=AB[:, j, 1:2])
                # bias corrections (tiny fp32 matvecs, off the critical path)
                qadj = stats.tile([P, 2, 1], f32)
                bva = stats.tile([P, 2, 1], f32)
                for jo in range(2):
                    tq = psd.tile([P, 1], f32, name="tq", tag="tiny", bufs=1)
                    for j in range(2):
                        nc.tensor.matmul(
                            tq, lhsT=wqk_e.bitcast(f32)[:, j, jo * P:(jo + 1) * P],
                            rhs=AB[:, j, 0:1], start=(j == 0), stop=(j == 1))
                    nc.scalar.copy(out=qadj[:, jo, :], in_=tq)
                    tv = psd.tile([P, 1], f32, name="tv", tag="tiny", bufs=1)
                    for j in range(2):
                        nc.tensor.matmul(
                            tv, lhsT=wv_e.bitcast(f32)[:, j, jo * P:(jo + 1) * P],
                            rhs=AB[:, j, 0:1], start=(j == 0), stop=(j == 1))
                    nc.scalar.copy(out=bva[:, jo, :], in_=tv)
                bqn = stats.tile([P, 2, 1], f32)
                nc.vector.tensor_sub(out=bqn, in0=bq, in1=qadj)
                wadj = stats.tile([P, 2, 1], f32)
                for jo in range(2):
                    tw = psd.tile([P, 1], f32, name="tw", tag="tiny", bufs=1)
                    for j in range(2):
                        nc.tensor.matmul(
                            tw, lhsT=wp.bitcast(f32)[:, j, jo * P:(jo + 1) * P],
                            rhs=bva[:, j, 0:1], start=(j == 0), stop=(j == 1))
                    nc.scalar.copy(out=wadj[:, jo, :], in_=tw)
                bpn = stats.tile([P, 2, 1], f32)
                nc.vector.tensor_sub(out=bpn, in0=bp, in1=wadj)
                # q (own half) and k (all columns)
                for jo in range(2):
                    for tt in range(NT):
                        sl = slice(tt * 512, (tt + 1) * 512)
                        ps = psd.tile([P, 512], f32, tag="mm")
                        for j in range(2):
                            nc.tensor.matmul(
                                ps, lhsT=wqk_e[:, j, jo * P:(jo + 1) * P],
                                rhs=xs[:, j, sl],
                                start=(j == 0), stop=(j == 1),
                            )
                        nc.vector.tensor_scalar_add(out=q_s[:, jo, sl],
                                                    in0=ps,
                                                    scalar1=bqn[:, jo, :])
                for jo in range(2):
                    for tt in range(KT):
                        sl = slice(tt * 512, (tt + 1) * 512)
                        ps = psd.tile([P, 512], f32, tag="mm")
                        for j in range(2):
                            nc.tensor.matmul(
                                ps, lhsT=wqk_e[:, j, C + jo * P:C + (jo + 1) * P],
                                rhs=xs[:, j, sl],
                                start=(j == 0), stop=(j == 1),
                            )
                        if tt % 2 == 0:
                            nc.scalar.copy(out=k_s[:, jo, sl], in_=ps)
                        else:
                            nc.vector.tensor_copy(out=k_s[:, jo, sl], in_=ps)
                # vpos[m, c]
                for mc in range(MC):
                    msl = slice(mc * P, (mc + 1) * P)
                    ps = psd.tile([P, 512], f32, tag="mm")
                    for j in range(2):
                        nc.tensor.matmul(
                            ps[:, 0:C], lhsT=xs[:, j, msl], rhs=wv_e[:, j, :],
                            start=(j == 0), stop=(j == 1),
                        )
                    if mc % 2 == 0:
                        nc.scalar.copy(out=v_s[:, mc, :], in_=ps[:, 0:C])
                    else:
                        nc.vector.tensor_copy(out=v_s[:, mc, :], in_=ps[:, 0:C])

            # ---- attention ----
            with (
                tc.tile_pool(name="psQK", bufs=3, space="PSUM") as psqk,
                tc.tile_pool(name="psAV", bufs=2, space="PSUM") as psav,
                tc.tile_pool(name="psSP", bufs=1, space="PSUM") as pssp,
            ):
                # Tile tails (S -> recip -> ha -> proj -> y) are emitted
                # INSIDE the next tile's mc loop: the PE executes in emission
                # order, so interleaving lets next-tile qk/av matmuls cover
                # the DVE recip/ha latency instead of stalling at boundaries.
                def tail_stage1(av0, av1, ea, st):
                    # S matmuls + recip + ha scale (PE 2 MMs + DVE work)
                    sps = pssp.tile([P, 512], f32, name="sps", tag="sp")
                    nc.tensor.matmul(sps, lhsT=ones_sq, rhs=ea[0],
                                     start=True, stop=False)
                    nc.tensor.matmul(sps, lhsT=ones_sq, rhs=ea[1],
                                     start=False, stop=True)
                    rb = rp.tile([P, 512], f32, name="rb", tag="rb")
                    nc.vector.reciprocal(out=rb, in_=sps)
                    ha = hap.tile([P, 2, 512], f32r, name="ha", tag="ha")
                    nc.vector.tensor_mul(out=ha[:, 0, :], in0=av0, in1=rb)
                    nc.vector.tensor_mul(out=ha[:, 1, :], in0=av1, in1=rb)
                    st["ha"] = ha

                def tail_stage2(st, psl):
                    ha = st["ha"]
                    yt = yp.tile([P, 2, 512], f32, name="yt", tag="yt")
                    for jo in range(2):
                        pp = pssp.tile([P, 512], f32, name="pp", tag="sp")
                        for j in range(2):
                            nc.tensor.matmul(
                                pp, lhsT=wp[:, j, jo * P:(jo + 1) * P],
                                rhs=ha[:, j, :],
                                start=(j == 0), stop=(j == 1),
                            )
                        nc.vector.scalar_tensor_tensor(
                            out=yt[:, jo, :], in0=pp, scalar=bpn[:, jo, :],
                            in1=xs.bitcast(f32)[:, jo, psl], op0=ALU.add, op1=ALU.add,
                        )
                    nc.sync.dma_start(out=yv[:, :, psl], in_=yt)

                pend = None
                for tt in range(NT):
                    sl = slice(tt * 512, (tt + 1) * 512)
                    # two interleaved exp-sum accumulators (halves the RAW chain)
                    ea = [accp.tile([P, 512], f32r, name=f"eacc{i}", tag=f"eacc{i}")
                          for i in range(2)]
                    nc.vector.memset(ea[0].bitcast(f32), 0.0)
                    nc.vector.memset(ea[1].bitcast(f32), 0.0)
                    av0 = psav.tile([P, 512], f32, name="av0", tag="av0")
                    av1 = psav.tile([P, 512], f32, name="av1", tag="av1")
                    # one-stage software pipeline: av[mc-1] runs while
                    # exp[mc] computes, so the PE never waits on the ACT.
                    ets = [None] * MC

                    def av_pair(mc, av0=av0, av1=av1, ea=ea, ets=ets):
                        et = ets[mc]
                        nc.tensor.matmul(av0, lhsT=v_s[:, mc, 0:P], rhs=et,
                                         start=(mc == 0), stop=(mc == MC - 1))
                        nc.tensor.matmul(av1, lhsT=v_s[:, mc, P:C], rhs=et,
                                         start=(mc == 0), stop=(mc == MC - 1))
                        acc = ea[mc % 2]
                        nc.vector.tensor_add(out=acc, in0=acc.bitcast(f32),
                                             in1=et.bitcast(f32))

                    for mc in range(MC):
                        msl = slice(mc * P, (mc + 1) * P)
                        qk = psqk.tile([P, 512], f32, name="qk", tag="qk")
                        for j in range(2):
                            nc.tensor.matmul(
                                qk, lhsT=k_s[:, j, msl], rhs=q_s[:, j, sl],
                                start=(j == 0), stop=(j == 1),
                            )
                        et = epool.tile([P, 512], f32r, name=f"et{mc % 6}",
                                        tag="et")
                        ets[mc] = et
                        nc.scalar.activation(out=et, in_=qk, func=AF.Exp)
                        if mc > 0:
                            av_pair(mc - 1)
                        if pend is not None:
                            if mc == 2:
                                tail_stage1(*pend[:3], pend[3])
                            elif mc == 8:
                                tail_stage2(pend[3], pend[4])
                                pend = None
                    av_pair(MC - 1)
                    pend = (av0, av1, ea, {}, sl)
                # last tile: split the tail per 256-col half so the DVE
                # recip/scale of half 1 overlaps the PE proj of half 0
                lav0, lav1, lea, _, lsl = pend
                sps = pssp.tile([P, 512], f32, name="sps_l", tag="sp")
                nc.tensor.matmul(sps, lhsT=ones_sq, rhs=lea[0],
                                 start=True, stop=False)
                nc.tensor.matmul(sps, lhsT=ones_sq, rhs=lea[1],
                                 start=False, stop=True)
                yt = yp.tile([P, 2, 512], f32, name="yt_l", tag="yt")
                for h in range(2):
                    hsl = slice(h * 256, (h + 1) * 256)
                    osl = slice(lsl.start + h * 256, lsl.start + (h + 1) * 256)
                    rbh = rp.tile([P, 256], f32, name=f"rbh{h}", tag="rb")
                    nc.vector.reciprocal(out=rbh, in_=sps[:, hsl])
                    hah = hap.tile([P, 2, 256], f32r, name=f"hah{h}", tag="ha")
                    nc.vector.tensor_mul(out=hah[:, 0, :], in0=lav0[:, hsl],
                                         in1=rbh)
                    nc.vector.tensor_mul(out=hah[:, 1, :], in0=lav1[:, hsl],
                                         in1=rbh)
                    for jo in range(2):
                        pp = psqk.tile([P, 256], f32, name="pp_l", tag="qk")
                        for j in range(2):
                            nc.tensor.matmul(
                                pp, lhsT=wp[:, j, jo * P:(jo + 1) * P],
                                rhs=hah[:, j, :],
                                start=(j == 0), stop=(j == 1),
                            )
                        nc.vector.scalar_tensor_tensor(
                            out=yt[:, jo, hsl], in0=pp, scalar=bpn[:, jo, :],
                            in1=xs.bitcast(f32)[:, jo, osl], op0=ALU.add, op1=ALU.add,
                        )
                nc.sync.dma_start(out=yv[:, :, lsl], in_=yt)

    nc.compile()
    return nc


def _get_prog():
    global _prog
    if _prog is None:
        _prog = _build_program()
    return _prog


def _host_prep(x, gn_w, gn_b, qkv_w, qkv_b, proj_w, proj_b):
    """Returns (shared input dict, per-core x list)."""
    x = np.asarray(x, dtype=np.float32)
    gn_w = np.asarray(gn_w, dtype=np.float32)
    gn_b = np.asarray(gn_b, dtype=np.float32)
    qkv_w = np.asarray(qkv_w, dtype=np.float32)
    qkv_b = np.asarray(qkv_b, dtype=np.float32)
    proj_w = np.asarray(proj_w, dtype=np.float32)
    proj_b = np.asarray(proj_b, dtype=np.float32)

    scale = 1.0 / np.sqrt(C).astype(np.float32)
    Wq = qkv_w[0:C] * gn_w[None, :] * scale
    bq_eff = (qkv_w[0:C] @ gn_b + qkv_b[0:C]) * scale
    Wk = qkv_w[C:2 * C] * gn_w[None, :]
    Wv = qkv_w[2 * C:3 * C] * gn_w[None, :]
    bv_eff = qkv_w[2 * C:3 * C] @ gn_b + qkv_b[2 * C:3 * C]
    bp_eff = proj_b + proj_w @ bv_eff

    wqk = np.concatenate([Wq.T, Wk.T], axis=1).astype(np.float32)  # [C, 2C]
    wv_h = np.ascontiguousarray(Wv.T, dtype=np.float32)
    wp_h = np.ascontiguousarray(proj_w.T, dtype=np.float32)

    cidx = np.arange(P)
    gm = np.zeros((P, 16), dtype=np.float32)
    gm[cidx, cidx // GSIZE] = 1.0 / GSIZE
    gt = np.zeros((16, P), dtype=np.float32)
    gt[cidx // GSIZE, cidx] = 1.0

    shared = {
        "onr": np.ones((P, P), dtype=np.float32),
        "wqk": wqk,
        "wv": wv_h,
        "wp": wp_h,
        "bq": bq_eff.reshape(C, 1).astype(np.float32),
        "bp": bp_eff.reshape(C, 1).astype(np.float32),
        "gm": gm,
        "gt": gt,
    }

    xf = x.reshape(B, C, N)
    xs_per_core = []
    for core in range(NCORES):
        b, half = core // 2, core % 2
        if half == 0:
            xc = xf[b]
        else:
            xc = np.concatenate([xf[b][:, NH:], xf[b][:, :NH]], axis=1)
        xs_per_core.append(np.ascontiguousarray(xc))
    return shared, xs_per_core


def run_sharded(inputs, trace=False, trace_kwargs=None):
    """Run the 8-core kernel. Returns (full_output, BassKernelResults)."""
    from concourse.bass_utils import run_bass_kernel_spmd

    nc = _get_prog()
    shared, xs_per_core = _host_prep(**inputs)
    in_maps = [{**shared, "x": xs_per_core[c]} for c in range(NCORES)]
    kw = {}
    if trace:
        kw["trace"] = True
        if trace_kwargs:
            kw["trace_kwargs"] = trace_kwargs
    res = run_bass_kernel_spmd(nc, in_maps, list(range(NCORES)), **kw)

    out = np.empty((B, C, N), dtype=np.float32)
    for core in range(NCORES):
        b, half = core // 2, core % 2
        yc = res.results[core]["y"]
        out[b][:, half * NH:(half + 1) * NH] = yc
    return out.reshape(B, C, HH, WW), res


def kernel(**inputs):
    out, _ = run_sharded(inputs)
    return out


# revision 31
# speedup vs baseline: 1.0278x; 1.0278x over previous
"""Trainium2 Bass kernel for nn_AttentionBlock (GroupNorm -> 1x1 qkv conv ->
softmax attention over N=HW -> 1x1 proj -> residual).

Sharding: 8 cores = 4 images x 2 query-column halves. Each core receives its
image column-permuted so its own 2048 query columns come first; attention is
permutation-invariant over key/value positions, so k/v use all 4096 columns
in permuted order. GroupNorm stats are computed on-chip per core (full image).

Math folding done on host (tiny O(C^2) numpy):
  - gn_w folded into qkv weight columns; gn_b folded into qkv biases.
  - 1/sqrt(C) score scale folded into Wq and bq.
  - k bias dropped entirely (adds a per-row constant to scores: softmax-invariant).
  - v bias folded into proj bias (softmax rows sum to 1): bp_eff = bp + Wp @ bv.
On-chip per core:
  h = (x - mean_g) * rstd_g              (per-channel affine from group stats)
  q = Wq^T-matmul(h) + bq  (cols 0:2048) ; k = Wk-matmul(h) (all cols)
  vpos[m, c] = Wv-matmul(h)              (position-major layout)
  per 512-col tile of q:  E[m, n] = exp(k_chunk^T q_tile) accumulated flash-style:
     av[c, n] += vpos_chunk^T E ;  eacc[n] += E (DVE)
  S = ones^T eacc (all-ones 128x128 matmul -> S replicated on all partitions)
  ha = av * (1/S) ; y = x_tile + Wp-matmul(ha) + bp_eff
"""

import numpy as np

B, C, HH, WW = 4, 256, 64, 64
N = HH * WW            # 4096
NH = N // 2            # 2048 query columns per core
GROUPS = 32
GSIZE = C // GROUPS    # 8
EPS = 1e-5
NCORES = 8
P = 128
NT = NH // 512         # 4 query tiles per core
MC = N // P            # 32 key chunks
KT = N // 512          # 8 column tiles for k

_prog = None


def _build_program():
    import concourse.bacc as bacc
    import concourse.tile as tile
    from concourse import mybir

    f32 = mybir.dt.float32
    f32r = mybir.dt.float32r
    AF = mybir.ActivationFunctionType
    ALU = mybir.AluOpType

    nc = bacc.Bacc("TRN2", target_bir_lowering=False, debug=False,
                   num_devices=NCORES)

    x_d = nc.dram_tensor("x", [C, N], f32, kind="ExternalInput").ap()
    wqk_d = nc.dram_tensor("wqk", [C, 2 * C], f32r, kind="ExternalInput").ap()
    wv_d = nc.dram_tensor("wv", [C, C], f32r, kind="ExternalInput").ap()
    wp_d = nc.dram_tensor("wp", [C, C], f32r, kind="ExternalInput").ap()
    bq_d = nc.dram_tensor("bq", [C, 1], f32, kind="ExternalInput").ap()
    bp_d = nc.dram_tensor("bp", [C, 1], f32, kind="ExternalInput").ap()
    gm_d = nc.dram_tensor("gm", [P, 16], f32, kind="ExternalInput").ap()
    gt_d = nc.dram_tensor("gt", [16, P], f32, kind="ExternalInput").ap()
    onr_d = nc.dram_tensor("onr", [P, P], f32r, kind="ExternalInput").ap()
    y_d = nc.dram_tensor("y", [C, NH], f32, kind="ExternalOutput").ap()

    xv = x_d.rearrange("(j p) n -> p j n", p=P)        # [128, 2, 4096]
    wqkv = wqk_d.rearrange("(j p) o -> p j o", p=P)    # [128, 2, 512]
    wvv = wv_d.rearrange("(j p) o -> p j o", p=P)
    wpv = wp_d.rearrange("(j p) o -> p j o", p=P)
    bqv = bq_d.rearrange("(j p) o -> p j o", p=P)      # [128, 2, 1]
    bpv = bp_d.rearrange("(j p) o -> p j o", p=P)
    yv = y_d.rearrange("(j p) n -> p j n", p=P)        # [128, 2, 2048]

    with tile.TileContext(nc) as tc:
        with (
            tc.tile_pool(name="big", bufs=1) as big,
            tc.tile_pool(name="wts", bufs=1) as wts,
            tc.tile_pool(name="stats", bufs=1) as stats,
            tc.tile_pool(name="epool", bufs=6) as epool,
            tc.tile_pool(name="acc", bufs=2) as accp,
            tc.tile_pool(name="rp", bufs=2) as rp,
            tc.tile_pool(name="hap", bufs=2) as hap,
            tc.tile_pool(name="yp", bufs=2) as yp,
        ):

            # PE warmup: dense dummy matmuls fill the x-DMA wait so the HAM
            # clock gate opens (K=8/8) before the real matmul stream starts.
            dummy = wts.tile([P, 512], f32)
            nc.vector.memset(dummy, 0.0)
            with tc.tile_pool(name="psW", bufs=1, space="PSUM") as psw:
                wps = psw.tile([P, 512], f32, tag="w")
                dr = dummy.bitcast(f32r)
                for _ in range(82):
                    nc.tensor.matmul(wps, lhsT=dr[:, 0:P], rhs=dr,
                                     start=True, stop=True)

            # ---- load x first (critical path), 3 parallel DMA queues ----
            xs = big.tile([P, 2, N], f32)
            dma_engs = [nc.sync, nc.gpsimd, nc.scalar, nc.sync]
            for j in range(2):
                for qd in range(4):
                    sl = slice(qd * 1024, (qd + 1) * 1024)
                    dma_engs[(j * 4 + qd) % 3].dma_start(
                        out=xs[:, j, sl], in_=xv[:, j, sl])

            # ---- weights / consts (off the critical path) ----
            wqk = wts.tile([P, 2, 2 * C], f32r)
            nc.gpsimd.dma_start(out=wqk, in_=wqkv)
            wv = wts.tile([P, 2, C], f32r)
            nc.scalar.dma_start(out=wv, in_=wvv)
            wp = wts.tile([P, 2, C], f32r)
            nc.scalar.dma_start(out=wp, in_=wpv)
            bq = wts.tile([P, 2, 1], f32)
            nc.sync.dma_start(out=bq, in_=bqv)
            bp = wts.tile([P, 2, 1], f32)
            nc.sync.dma_start(out=bp, in_=bpv)
            gm = wts.tile([P, 16], f32)
            nc.sync.dma_start(out=gm, in_=gm_d)
            gt = wts.tile([16, P], f32)
            nc.sync.dma_start(out=gt, in_=gt_d)
            ones_sq = wts.tile([P, P], f32r)
            nc.sync.dma_start(out=ones_sq, in_=onr_d)
            eps_t = wts.tile([16, 1], f32)
            nc.vector.memset(eps_t, EPS)

            # ---- group stats ----
            AB = stats.tile([P, 2, 2], f32)  # per-channel (mean, rstd)
            with tc.tile_pool(name="psStat", bufs=1, space="PSUM") as psst:
                for j in range(2):
                    st6 = stats.tile([P, 8, 6], f32, tag="st6")
                    xsr = xs[:, j, :].rearrange("p (s f) -> p s f", f=512)
                    for sg in range(8):
                        nc.vector.bn_stats(out=st6[:, sg, :], in_=xsr[:, sg, :])
                    mv = stats.tile([P, 2], f32, tag="mv")
                    nc.vector.bn_aggr(out=mv, in_=st6)
                    # t2 = (mean, var + mean^2)
                    t2 = stats.tile([P, 2], f32, tag="t2")
                    nc.vector.tensor_copy(out=t2[:, 0:1], in_=mv[:, 0:1])
                    nc.vector.scalar_tensor_tensor(
                        out=t2[:, 1:2], in0=mv[:, 0:1], scalar=mv[:, 0:1],
                        in1=mv[:, 1:2], op0=ALU.mult, op1=ALU.add,
                    )
                    gagg = psst.tile([16, 2], f32, tag="gagg")
                    nc.tensor.matmul(gagg, lhsT=gm, rhs=t2, start=True, stop=True)
                    # grs = (gmean, rstd)
                    grs = stats.tile([16, 2], f32, tag="grs")
                    nc.scalar.copy(out=grs[:, 0:1], in_=gagg[:, 0:1])
                    sq = stats.tile([16, 1], f32, tag="sq")
                    nc.scalar.square(out=sq, in_=gagg[:, 0:1])
                    var = stats.tile([16, 1], f32, tag="var")
                    nc.vector.tensor_sub(out=var, in0=gagg[:, 1:2], in1=sq)
                    nc.scalar.activation(out=var, in_=var, func=AF.Sqrt,
                                         bias=eps_t, scale=1.0)
                    nc.vector.reciprocal(out=grs[:, 1:2], in_=var)
                    gb = psst.tile([P, 2], f32, tag="gb")
                    nc.tensor.matmul(gb, lhsT=gt, rhs=grs, start=True, stop=True)
                    nc.scalar.copy(out=AB[:, j, :], in_=gb)

            # bridge the PE clock gate through the normalize (DVE) phase
            with tc.tile_pool(name="psW2", bufs=1, space="PSUM") as psw2:
                wps2 = psw2.tile([P, 512], f32, tag="w2")
                dr2 = dummy.bitcast(f32r)
                for _ in range(25):
                    nc.tensor.matmul(wps2, lhsT=dr2[:, 0:P], rhs=dr2,
                                     start=True, stop=True)

            # ---- qkv ----
            q_s = big.tile([P, 2, NH], f32r)
            k_s = big.tile([P, 2, N], f32r)
            v_s = big.tile([P, MC, C], f32r)
            with (
                tc.tile_pool(name="hp", bufs=1) as hp,
                tc.tile_pool(name="psD", bufs=4, space="PSUM") as psd,
            ):
                hs = hp.tile([P, 2, N], f32r)
                for j in range(2):
                    for nd in range(4):
                        ns = slice(nd * 1024, (nd + 1) * 1024)
                        nc.vector.tensor_scalar(
                            out=hs[:, j, ns], in0=xs[:, j, ns],
                            scalar1=AB[:, j, 0:1], scalar2=AB[:, j, 1:2],
                            op0=ALU.subtract, op1=ALU.mult,
                        )
                # q (own half) and k (all columns)
                for jo in range(2):
                    for tt in range(NT):
                        sl = slice(tt * 512, (tt + 1) * 512)
                        ps = psd.tile([P, 512], f32, tag="mm")
                        for j in range(2):
                            nc.tensor.matmul(
                                ps, lhsT=wqk[:, j, jo * P:(jo + 1) * P],
                                rhs=hs[:, j, sl],
                                start=(j == 0), stop=(j == 1),
                            )
                        nc.vector.tensor_scalar_add(out=q_s[:, jo, sl],
                                                    in0=ps,
                                                    scalar1=bq[:, jo, :])
                for jo in range(2):
                    for tt in range(KT):
                        sl = slice(tt * 512, (tt + 1) * 512)
                        ps = psd.tile([P, 512], f32, tag="mm")
                        for j in range(2):
                            nc.tensor.matmul(
                                ps, lhsT=wqk[:, j, C + jo * P:C + (jo + 1) * P],
                                rhs=hs[:, j, sl],
                                start=(j == 0), stop=(j == 1),
                            )
                        if tt % 2 == 0:
                            nc.scalar.copy(out=k_s[:, jo, sl], in_=ps)
                        else:
                            nc.vector.tensor_copy(out=k_s[:, jo, sl], in_=ps)
                # vpos[m, c]
                for mc in range(MC):
                    msl = slice(mc * P, (mc + 1) * P)
                    ps = psd.tile([P, 512], f32, tag="mm")
                    for j in range(2):
                        nc.tensor.matmul(
                            ps[:, 0:C], lhsT=hs[:, j, msl], rhs=wv[:, j, :],
                            start=(j == 0), stop=(j == 1),
                        )
                    if mc % 2 == 0:
                        nc.scalar.copy(out=v_s[:, mc, :], in_=ps[:, 0:C])
                    else:
                        nc.vector.tensor_copy(out=v_s[:, mc, :], in_=ps[:, 0:C])

            # ---- attention ----
            with (
                tc.tile_pool(name="psQK", bufs=3, space="PSUM") as psqk,
                tc.tile_pool(name="psAV", bufs=2, space="PSUM") as psav,
                tc.tile_pool(name="psSP", bufs=1, space="PSUM") as pssp,
            ):
                # Tile tails (S -> recip -> ha -> proj -> y) are emitted
                # INSIDE the next tile's mc loop: the PE executes in emission
                # order, so interleaving lets next-tile qk/av matmuls cover
                # the DVE recip/ha latency instead of stalling at boundaries.
                def tail_stage1(av0, av1, ea, st):
                    # S matmuls + recip + ha scale (PE 2 MMs + DVE work)
                    sps = pssp.tile([P, 512], f32, name="sps", tag="sp")
                    nc.tensor.matmul(sps, lhsT=ones_sq, rhs=ea[0],
                                     start=True, stop=False)
                    nc.tensor.matmul(sps, lhsT=ones_sq, rhs=ea[1],
                                     start=False, stop=True)
                    rb = rp.tile([P, 512], f32, name="rb", tag="rb")
                    nc.vector.reciprocal(out=rb, in_=sps)
                    ha = hap.tile([P, 2, 512], f32r, name="ha", tag="ha")
                    nc.vector.tensor_mul(out=ha[:, 0, :], in0=av0, in1=rb)
                    nc.vector.tensor_mul(out=ha[:, 1, :], in0=av1, in1=rb)
                    st["ha"] = ha

                def tail_stage2(st, psl):
                    ha = st["ha"]
                    yt = yp.tile([P, 2, 512], f32, name="yt", tag="yt")
                    for jo in range(2):
                        pp = pssp.tile([P, 512], f32, name="pp", tag="sp")
                        for j in range(2):
                            nc.tensor.matmul(
                                pp, lhsT=wp[:, j, jo * P:(jo + 1) * P],
                                rhs=ha[:, j, :],
                                start=(j == 0), stop=(j == 1),
                            )
                        nc.vector.scalar_tensor_tensor(
                            out=yt[:, jo, :], in0=pp, scalar=bp[:, jo, :],
                            in1=xs[:, jo, psl], op0=ALU.add, op1=ALU.add,
                        )
                    nc.sync.dma_start(out=yv[:, :, psl], in_=yt)

                pend = None
                for tt in range(NT):
                    sl = slice(tt * 512, (tt + 1) * 512)
                    # two interleaved exp-sum accumulators (halves the RAW chain)
                    ea = [accp.tile([P, 512], f32r, name=f"eacc{i}", tag=f"eacc{i}")
                          for i in range(2)]
                    nc.vector.memset(ea[0].bitcast(f32), 0.0)
                    nc.vector.memset(ea[1].bitcast(f32), 0.0)
                    av0 = psav.tile([P, 512], f32, name="av0", tag="av0")
                    av1 = psav.tile([P, 512], f32, name="av1", tag="av1")
                    # one-stage software pipeline: av[mc-1] runs while
                    # exp[mc] computes, so the PE never waits on the ACT.
                    ets = [None] * MC

                    def av_pair(mc, av0=av0, av1=av1, ea=ea, ets=ets):
                        et = ets[mc]
                        nc.tensor.matmul(av0, lhsT=v_s[:, mc, 0:P], rhs=et,
                                         start=(mc == 0), stop=(mc == MC - 1))
                        nc.tensor.matmul(av1, lhsT=v_s[:, mc, P:C], rhs=et,
                                         start=(mc == 0), stop=(mc == MC - 1))
                        acc = ea[mc % 2]
                        nc.vector.tensor_add(out=acc, in0=acc.bitcast(f32),
                                             in1=et.bitcast(f32))

                    for mc in range(MC):
                        msl = slice(mc * P, (mc + 1) * P)
                        qk = psqk.tile([P, 512], f32, name="qk", tag="qk")
                        for j in range(2):
                            nc.tensor.matmul(
                                qk, lhsT=k_s[:, j, msl], rhs=q_s[:, j, sl],
                                start=(j == 0), stop=(j == 1),
                            )
                        et = epool.tile([P, 512], f32r, name=f"et{mc % 6}",
                                        tag="et")
                        ets[mc] = et
                        nc.scalar.activation(out=et, in_=qk, func=AF.Exp)
                        if mc > 0:
                            av_pair(mc - 1)
                        if pend is not None:
                            if mc == 2:
                                tail_stage1(*pend[:3], pend[3])
                            elif mc == 8:
                                tail_stage2(pend[3], pend[4])
                                pend = None
                    av_pair(MC - 1)
                    pend = (av0, av1, ea, {}, sl)
                # last tile: split the tail per 256-col half so the DVE
                # recip/scale of half 1 overlaps the PE proj of half 0
                lav0, lav1, lea, _, lsl = pend
                sps = pssp.tile([P, 512], f32, name="sps_l", tag="sp")
                nc.tensor.matmul(sps, lhsT=ones_sq, rhs=lea[0],
                                 start=True, stop=False)
                nc.tensor.matmul(sps, lhsT=ones_sq, rhs=lea[1],
                                 start=False, stop=True)
                yt = yp.tile([P, 2, 512], f32, name="yt_l", tag="yt")
                for h in range(2):
                    hsl = slice(h * 256, (h + 1) * 256)
                    osl = slice(lsl.start + h * 256, lsl.start + (h + 1) * 256)
                    rbh = rp.tile([P, 256], f32, name=f"rbh{h}", tag="rb")
                    nc.vector.reciprocal(out=rbh, in_=sps[:, hsl])
                    hah = hap.tile([P, 2, 256], f32r, name=f"hah{h}", tag="ha")
                    nc.vector.tensor_mul(out=hah[:, 0, :], in0=lav0[:, hsl],
                                         in1=rbh)
                    nc.vector.tensor_mul(out=hah[:, 1, :], in0=lav1[:, hsl],
                                         in1=rbh)
                    for jo in range(2):
                        pp = psqk.tile([P, 256], f32, name="pp_l", tag="qk")
                        for j in range(2):
                            nc.tensor.matmul(
                                pp, lhsT=wp[:, j, jo * P:(jo + 1) * P],
                                rhs=hah[:, j, :],
                                start=(j == 0), stop=(j == 1),
                            )
                        nc.vector.scalar_tensor_tensor(
                            out=yt[:, jo, hsl], in0=pp, scalar=bp[:, jo, :],
                            in1=xs[:, jo, osl], op0=ALU.add, op1=ALU.add,
                        )
                nc.sync.dma_start(out=yv[:, :, lsl], in_=yt)

    nc.compile()
    return nc


def _get_prog():
    global _prog
    if _prog is None:
        _prog = _build_program()
    return _prog


def _host_prep(x, gn_w, gn_b, qkv_w, qkv_b, proj_w, proj_b):
    """Returns (shared input dict, per-core x list)."""
    x = np.asarray(x, dtype=np.float32)
    gn_w = np.asarray(gn_w, dtype=np.float32)
    gn_b = np.asarray(gn_b, dtype=np.float32)
    qkv_w = np.asarray(qkv_w, dtype=np.float32)
    qkv_b = np.asarray(qkv_b, dtype=np.float32)
    proj_w = np.asarray(proj_w, dtype=np.float32)
    proj_b = np.asarray(proj_b, dtype=np.float32)

    scale = 1.0 / np.sqrt(C).astype(np.float32)
    Wq = qkv_w[0:C] * gn_w[None, :] * scale
    bq_eff = (qkv_w[0:C] @ gn_b + qkv_b[0:C]) * scale
    Wk = qkv_w[C:2 * C] * gn_w[None, :]
    Wv = qkv_w[2 * C:3 * C] * gn_w[None, :]
    bv_eff = qkv_w[2 * C:3 * C] @ gn_b + qkv_b[2 * C:3 * C]
    bp_eff = proj_b + proj_w @ bv_eff

    wqk = np.concatenate([Wq.T, Wk.T], axis=1).astype(np.float32)  # [C, 2C]
    wv_h = np.ascontiguousarray(Wv.T, dtype=np.float32)
    wp_h = np.ascontiguousarray(proj_w.T, dtype=np.float32)

    cidx = np.arange(P)
    gm = np.zeros((P, 16), dtype=np.float32)
    gm[cidx, cidx // GSIZE] = 1.0 / GSIZE
    gt = np.zeros((16, P), dtype=np.float32)
    gt[cidx // GSIZE, cidx] = 1.0

    shared = {
        "onr": np.ones((P, P), dtype=np.float32),
        "wqk": wqk,
        "wv": wv_h,
        "wp": wp_h,
        "bq": bq_eff.reshape(C, 1).astype(np.float32),
        "bp": bp_eff.reshape(C, 1).astype(np.float32),
        "gm": gm,
        "gt": gt,
    }

    xf = x.reshape(B, C, N)
    xs_per_core = []
    for core in range(NCORES):
        b, half = core // 2, core % 2
        if half == 0:
            xc = xf[b]
        else:
            xc = np.concatenate([xf[b][:, NH:], xf[b][:, :NH]], axis=1)
        xs_per_core.append(np.ascontiguousarray(xc))
    return shared, xs_per_core


def run_sharded(inputs, trace=False, trace_kwargs=None):
    """Run the 8-core kernel. Returns (full_output, BassKernelResults)."""
    from concourse.bass_utils import run_bass_kernel_spmd

    nc = _get_prog()
    shared, xs_per_core = _host_prep(**inputs)
    in_maps = [{**shared, "x": xs_per_core[c]} for c in range(NCORES)]
    kw = {}
    if trace:
        kw["trace"] = True
        if trace_kwargs:
            kw["trace_kwargs"] = trace_kwargs
    res = run_bass_kernel_spmd(nc, in_maps, list(range(NCORES)), **kw)

    out = np.empty((B, C, N), dtype=np.float32)
    for core in range(NCORES):
        b, half = core // 2, core % 2
        yc = res.results[core]["y"]
        out[b][:, half * NH:(half + 1) * NH] = yc
    return out.reshape(B, C, HH, WW), res


def kernel(**inputs):
    out, _ = run_sharded(inputs)
    return out


# revision 33
# speedup vs baseline: 1.0289x; 1.0012x over previous
"""Trainium2 Bass kernel for nn_AttentionBlock (GroupNorm -> 1x1 qkv conv ->
softmax attention over N=HW -> 1x1 proj -> residual).

Sharding: 8 cores = 4 images x 2 query-column halves. Each core receives its
image column-permuted so its own 2048 query columns come first; attention is
permutation-invariant over key/value positions, so k/v use all 4096 columns
in permuted order. GroupNorm stats are computed on-chip per core (full image).

Math folding done on host (tiny O(C^2) numpy):
  - gn_w folded into qkv weight columns; gn_b folded into qkv biases.
  - 1/sqrt(C) score scale folded into Wq and bq.
  - k bias dropped entirely (adds a per-row constant to scores: softmax-invariant).
  - v bias folded into proj bias (softmax rows sum to 1): bp_eff = bp + Wp @ bv.
On-chip per core:
  h = (x - mean_g) * rstd_g              (per-channel affine from group stats)
  q = Wq^T-matmul(h) + bq  (cols 0:2048) ; k = Wk-matmul(h) (all cols)
  vpos[m, c] = Wv-matmul(h)              (position-major layout)
  per 512-col tile of q:  E[m, n] = exp(k_chunk^T q_tile) accumulated flash-style:
     av[c, n] += vpos_chunk^T E ;  eacc[n] += E (DVE)
  S = ones^T eacc (all-ones 128x128 matmul -> S replicated on all partitions)
  ha = av * (1/S) ; y = x_tile + Wp-matmul(ha) + bp_eff
"""

import numpy as np

B, C, HH, WW = 4, 256, 64, 64
N = HH * WW            # 4096
NH = N // 2            # 2048 query columns per core
GROUPS = 32
GSIZE = C // GROUPS    # 8
EPS = 1e-5
NCORES = 8
P = 128
NT = NH // 512         # 4 query tiles per core
MC = N // P            # 32 key chunks
KT = N // 512          # 8 column tiles for k

_prog = None


def _build_program():
    import concourse.bacc as bacc
    import concourse.tile as tile
    from concourse import mybir

    f32 = mybir.dt.float32
    f32r = mybir.dt.float32r
    AF = mybir.ActivationFunctionType
    ALU = mybir.AluOpType

    nc = bacc.Bacc("TRN2", target_bir_lowering=False, debug=False,
                   num_devices=NCORES)

    x_d = nc.dram_tensor("x", [C, N], f32, kind="ExternalInput").ap()
    wqk_d = nc.dram_tensor("wqk", [C, 2 * C], f32r, kind="ExternalInput").ap()
    wv_d = nc.dram_tensor("wv", [C, C], f32r, kind="ExternalInput").ap()
    wp_d = nc.dram_tensor("wp", [C, C], f32r, kind="ExternalInput").ap()
    bq_d = nc.dram_tensor("bq", [C, 1], f32, kind="ExternalInput").ap()
    bp_d = nc.dram_tensor("bp", [C, 1], f32, kind="ExternalInput").ap()
    gm_d = nc.dram_tensor("gm", [P, 16], f32, kind="ExternalInput").ap()
    gt_d = nc.dram_tensor("gt", [16, P], f32, kind="ExternalInput").ap()
    onr_d = nc.dram_tensor("onr", [P, P], f32r, kind="ExternalInput").ap()
    y_d = nc.dram_tensor("y", [C, NH], f32, kind="ExternalOutput").ap()

    xv = x_d.rearrange("(j p) n -> p j n", p=P)        # [128, 2, 4096]
    wqkv = wqk_d.rearrange("(j p) o -> p j o", p=P)    # [128, 2, 512]
    wvv = wv_d.rearrange("(j p) o -> p j o", p=P)
    wpv = wp_d.rearrange("(j p) o -> p j o", p=P)
    bqv = bq_d.rearrange("(j p) o -> p j o", p=P)      # [128, 2, 1]
    bpv = bp_d.rearrange("(j p) o -> p j o", p=P)
    yv = y_d.rearrange("(j p) n -> p j n", p=P)        # [128, 2, 2048]

    with tile.TileContext(nc) as tc:
        with (
            tc.tile_pool(name="big", bufs=1) as big,
            tc.tile_pool(name="wts", bufs=1) as wts,
            tc.tile_pool(name="stats", bufs=1) as stats,
            tc.tile_pool(name="epool", bufs=6) as epool,
            tc.tile_pool(name="acc", bufs=2) as accp,
            tc.tile_pool(name="rp", bufs=2) as rp,
            tc.tile_pool(name="hap", bufs=2) as hap,
            tc.tile_pool(name="yp", bufs=2) as yp,
        ):

            # PE warmup: dense dummy matmuls fill the x-DMA wait so the HAM
            # clock gate opens (K=8/8) before the real matmul stream starts.
            dummy = wts.tile([P, 512], f32)
            nc.vector.memset(dummy, 0.0)
            with tc.tile_pool(name="psW", bufs=1, space="PSUM") as psw:
                wps = psw.tile([P, 512], f32, tag="w")
                dr = dummy.bitcast(f32r)
                for _ in range(82):
                    nc.tensor.matmul(wps, lhsT=dr[:, 0:P], rhs=dr,
                                     start=True, stop=True)

            # ---- load x first (critical path), 3 parallel DMA queues ----
            xs = big.tile([P, 2, N], f32)
            dma_engs = [nc.sync, nc.gpsimd, nc.scalar, nc.sync]
            for j in range(2):
                for qd in range(4):
                    sl = slice(qd * 1024, (qd + 1) * 1024)
                    dma_engs[(j * 4 + qd) % 3].dma_start(
                        out=xs[:, j, sl], in_=xv[:, j, sl])

            # ---- weights / consts (off the critical path) ----
            wqk = wts.tile([P, 2, 2 * C], f32r)
            nc.gpsimd.dma_start(out=wqk, in_=wqkv)
            wv = wts.tile([P, 2, C], f32r)
            nc.scalar.dma_start(out=wv, in_=wvv)
            wp = wts.tile([P, 2, C], f32r)
            nc.scalar.dma_start(out=wp, in_=wpv)
            bq = wts.tile([P, 2, 1], f32)
            nc.sync.dma_start(out=bq, in_=bqv)
            bp = wts.tile([P, 2, 1], f32)
            nc.sync.dma_start(out=bp, in_=bpv)
            gm = wts.tile([P, 16], f32)
            nc.sync.dma_start(out=gm, in_=gm_d)
            gt = wts.tile([16, P], f32)
            nc.sync.dma_start(out=gt, in_=gt_d)
            ones_sq = wts.tile([P, P], f32r)
            nc.sync.dma_start(out=ones_sq, in_=onr_d)
            eps_t = wts.tile([16, 1], f32)
            nc.vector.memset(eps_t, EPS)

            # ---- group stats ----
            AB = stats.tile([P, 2, 2], f32)  # per-channel (mean, rstd)
            with tc.tile_pool(name="psStat", bufs=1, space="PSUM") as psst:
                for j in range(2):
                    st6 = stats.tile([P, 8, 6], f32, tag="st6")
                    xsr = xs[:, j, :].rearrange("p (s f) -> p s f", f=512)
                    for sg in range(8):
                        nc.vector.bn_stats(out=st6[:, sg, :], in_=xsr[:, sg, :])
                    mv = stats.tile([P, 2], f32, tag="mv")
                    nc.vector.bn_aggr(out=mv, in_=st6)
                    # t2 = (mean, var + mean^2)
                    t2 = stats.tile([P, 2], f32, tag="t2")
                    nc.vector.tensor_copy(out=t2[:, 0:1], in_=mv[:, 0:1])
                    nc.vector.scalar_tensor_tensor(
                        out=t2[:, 1:2], in0=mv[:, 0:1], scalar=mv[:, 0:1],
                        in1=mv[:, 1:2], op0=ALU.mult, op1=ALU.add,
                    )
                    gagg = psst.tile([16, 2], f32, tag="gagg")
                    nc.tensor.matmul(gagg, lhsT=gm, rhs=t2, start=True, stop=True)
                    # grs = (gmean, rstd)
                    grs = stats.tile([16, 2], f32, tag="grs")
                    nc.scalar.copy(out=grs[:, 0:1], in_=gagg[:, 0:1])
                    sq = stats.tile([16, 1], f32, tag="sq")
                    nc.scalar.square(out=sq, in_=gagg[:, 0:1])
                    var = stats.tile([16, 1], f32, tag="var")
                    nc.vector.tensor_sub(out=var, in0=gagg[:, 1:2], in1=sq)
                    nc.scalar.activation(out=var, in_=var, func=AF.Sqrt,
                                         bias=eps_t, scale=1.0)
                    nc.vector.reciprocal(out=grs[:, 1:2], in_=var)
                    gb = psst.tile([P, 2], f32, tag="gb")
                    nc.tensor.matmul(gb, lhsT=gt, rhs=grs, start=True, stop=True)
                    nc.scalar.copy(out=AB[:, j, :], in_=gb)

            # bridge the PE clock gate through the normalize (DVE) phase
            with tc.tile_pool(name="psW2", bufs=1, space="PSUM") as psw2:
                wps2 = psw2.tile([P, 512], f32, tag="w2")
                dr2 = dummy.bitcast(f32r)
                for _ in range(25):
                    nc.tensor.matmul(wps2, lhsT=dr2[:, 0:P], rhs=dr2,
                                     start=True, stop=True)

            # ---- qkv ----
            q_s = big.tile([P, 2, NH], f32r)
            k_s = big.tile([P, 2, N], f32r)
            v_s = big.tile([P, MC, C], f32r)
            with (
                tc.tile_pool(name="hp", bufs=1) as hp,
                tc.tile_pool(name="psD", bufs=4, space="PSUM") as psd,
            ):
                hs = hp.tile([P, 2, N], f32r)
                for j in range(2):
                    for nd in range(4):
                        ns = slice(nd * 1024, (nd + 1) * 1024)
                        nc.vector.tensor_scalar(
                            out=hs[:, j, ns], in0=xs[:, j, ns],
                            scalar1=AB[:, j, 0:1], scalar2=AB[:, j, 1:2],
                            op0=ALU.subtract, op1=ALU.mult,
                        )
                # q (own half) and k (all columns)
                for jo in range(2):
                    for tt in range(NT):
                        sl = slice(tt * 512, (tt + 1) * 512)
                        ps = psd.tile([P, 512], f32, tag="mm")
                        for j in range(2):
                            nc.tensor.matmul(
                                ps, lhsT=wqk[:, j, jo * P:(jo + 1) * P],
                                rhs=hs[:, j, sl],
                                start=(j == 0), stop=(j == 1),
                            )
                        nc.vector.tensor_scalar_add(out=q_s[:, jo, sl],
                                                    in0=ps,
                                                    scalar1=bq[:, jo, :])
                for jo in range(2):
                    for tt in range(KT):
                        sl = slice(tt * 512, (tt + 1) * 512)
                        ps = psd.tile([P, 512], f32, tag="mm")
                        for j in range(2):
                            nc.tensor.matmul(
                                ps, lhsT=wqk[:, j, C + jo * P:C + (jo + 1) * P],
                                rhs=hs[:, j, sl],
                                start=(j == 0), stop=(j == 1),
                            )
                        if tt % 2 == 0:
                            nc.scalar.copy(out=k_s[:, jo, sl], in_=ps)
                        else:
                            nc.vector.tensor_copy(out=k_s[:, jo, sl], in_=ps)
                # vpos[m, c]
                for mc in range(MC):
                    msl = slice(mc * P, (mc + 1) * P)
                    ps = psd.tile([P, 512], f32, tag="mm")
                    for j in range(2):
                        nc.tensor.matmul(
                            ps[:, 0:C], lhsT=hs[:, j, msl], rhs=wv[:, j, :],
                            start=(j == 0), stop=(j == 1),
                        )
                    if mc % 2 == 0:
                        nc.scalar.copy(out=v_s[:, mc, :], in_=ps[:, 0:C])
                    else:
                        nc.vector.tensor_copy(out=v_s[:, mc, :], in_=ps[:, 0:C])

            # ---- attention ----
            with (
                tc.tile_pool(name="psQK", bufs=3, space="PSUM") as psqk,
                tc.tile_pool(name="psAV", bufs=2, space="PSUM") as psav,
                tc.tile_pool(name="psSP", bufs=1, space="PSUM") as pssp,
            ):
                # Tile tails (S -> recip -> ha -> proj -> y) are emitted
                # INSIDE the next tile's mc loop: the PE executes in emission
                # order, so interleaving lets next-tile qk/av matmuls cover
                # the DVE recip/ha latency instead of stalling at boundaries.
                def tail_stage1(av0, av1, ea, st):
                    # S matmuls + recip + ha scale (PE 2 MMs + DVE work)
                    sps = pssp.tile([P, 512], f32, name="sps", tag="sp")
                    nc.tensor.matmul(sps, lhsT=ones_sq, rhs=ea[0],
                                     start=True, stop=False)
                    nc.tensor.matmul(sps, lhsT=ones_sq, rhs=ea[1],
                                     start=False, stop=True)
                    rb = rp.tile([P, 512], f32, name="rb", tag="rb")
                    nc.vector.reciprocal(out=rb, in_=sps)
                    ha = hap.tile([P, 2, 512], f32r, name="ha", tag="ha")
                    nc.vector.tensor_mul(out=ha[:, 0, :], in0=av0, in1=rb)
                    nc.vector.tensor_mul(out=ha[:, 1, :], in0=av1, in1=rb)
                    st["ha"] = ha

                def tail_stage2(st, psl):
                    ha = st["ha"]
                    yt = yp.tile([P, 2, 512], f32, name="yt", tag="yt")
                    for jo in range(2):
                        pp = pssp.tile([P, 512], f32, name="pp", tag="sp")
                        for j in range(2):
                            nc.tensor.matmul(
                                pp, lhsT=wp[:, j, jo * P:(jo + 1) * P],
                                rhs=ha[:, j, :],
                                start=(j == 0), stop=(j == 1),
                            )
                        nc.vector.scalar_tensor_tensor(
                            out=yt[:, jo, :], in0=pp, scalar=bp[:, jo, :],
                            in1=xs[:, jo, psl], op0=ALU.add, op1=ALU.add,
                        )
                    nc.sync.dma_start(out=yv[:, :, psl], in_=yt)

                pend = None
                for tt in range(NT):
                    sl = slice(tt * 512, (tt + 1) * 512)
                    # two interleaved exp-sum accumulators (halves the RAW chain)
                    ea = [accp.tile([P, 512], f32r, name=f"eacc{i}", tag=f"eacc{i}")
                          for i in range(2)]
                    nc.vector.memset(ea[0].bitcast(f32), 0.0)
                    nc.vector.memset(ea[1].bitcast(f32), 0.0)
                    av0 = psav.tile([P, 512], f32, name="av0", tag="av0")
                    av1 = psav.tile([P, 512], f32, name="av1", tag="av1")
                    # one-stage software pipeline: av[mc-1] runs while
                    # exp[mc] computes, so the PE never waits on the ACT.
                    ets = [None] * MC

                    def av_pair(mc, av0=av0, av1=av1, ea=ea, ets=ets):
                        et = ets[mc]
                        nc.tensor.matmul(av0, lhsT=v_s[:, mc, 0:P], rhs=et,
                                         start=(mc == 0), stop=(mc == MC - 1))
                        nc.tensor.matmul(av1, lhsT=v_s[:, mc, P:C], rhs=et,
                                         start=(mc == 0), stop=(mc == MC - 1))
                        acc = ea[mc % 2]
                        nc.vector.tensor_add(out=acc, in0=acc.bitcast(f32),
                                             in1=et.bitcast(f32))

                    for mc in range(MC):
                        msl = slice(mc * P, (mc + 1) * P)
                        qk = psqk.tile([P, 512], f32, name="qk", tag="qk")
                        for j in range(2):
                            nc.tensor.matmul(
                                qk, lhsT=k_s[:, j, msl], rhs=q_s[:, j, sl],
                                start=(j == 0), stop=(j == 1),
                            )
                        et = epool.tile([P, 512], f32r, name=f"et{mc % 6}",
                                        tag="et")
                        ets[mc] = et
                        nc.scalar.activation(out=et, in_=qk, func=AF.Exp)
                        if mc > 0:
                            av_pair(mc - 1)
                        if pend is not None:
                            if mc == 2:
                                tail_stage1(*pend[:3], pend[3])
                            elif mc == 8:
                                tail_stage2(pend[3], pend[4])
                                pend = None
                    av_pair(MC - 1)
                    pend = (av0, av1, ea, {}, sl)
                # last tile: split the tail per 256-col half so the DVE
                # recip/scale of half 1 overlaps the PE proj of half 0
                lav0, lav1, lea, _, lsl = pend
                sps = pssp.tile([P, 512], f32, name="sps_l", tag="sp")
                nc.tensor.matmul(sps, lhsT=ones_sq, rhs=lea[0],
                                 start=True, stop=False)
                nc.tensor.matmul(sps, lhsT=ones_sq, rhs=lea[1],
                                 start=False, stop=True)
                yt = yp.tile([P, 2, 512], f32, name="yt_l", tag="yt")
                for h in range(2):
                    hsl = slice(h * 256, (h + 1) * 256)
                    osl = slice(lsl.start + h * 256, lsl.start + (h + 1) * 256)
                    rbh = rp.tile([P, 256], f32, name=f"rbh{h}", tag="rb")
                    nc.vector.reciprocal(out=rbh, in_=sps[:, hsl])
                    hah = hap.tile([P, 2, 256], f32r, name=f"hah{h}", tag="ha")
                    nc.vector.tensor_mul(out=hah[:, 0, :], in0=lav0[:, hsl],
                                         in1=rbh)
                    nc.vector.tensor_mul(out=hah[:, 1, :], in0=lav1[:, hsl],
                                         in1=rbh)
                    for jo in range(2):
                        pp = psqk.tile([P, 256], f32, name="pp_l", tag="qk")
                        for j in range(2):
                            nc.tensor.matmul(
                                pp, lhsT=wp[:, j, jo * P:(jo + 1) * P],
                                rhs=hah[:, j, :],
                                start=(j == 0), stop=(j == 1),
                            )
                        nc.vector.scalar_tensor_tensor(
                            out=yt[:, jo, hsl], in0=pp, scalar=bp[:, jo, :],
                            in1=xs[:, jo, osl], op0=ALU.add, op1=ALU.add,
                        )
                nc.sync.dma_start(out=yv[:, :, lsl], in_=yt)

    nc.compile()
    return nc


def _get_prog():
    global _prog
    if _prog is None:
        _prog = _build_program()
    return _prog


def _host_prep(x, gn_w, gn_b, qkv_w, qkv_b, proj_w, proj_b):
    """Returns (shared input dict, per-core x list)."""
    x = np.asarray(x, dtype=np.float32)
    gn_w = np.asarray(gn_w, dtype=np.float32)
    gn_b = np.asarray(gn_b, dtype=np.float32)
    qkv_w = np.asarray(qkv_w, dtype=np.float32)
    qkv_b = np.asarray(qkv_b, dtype=np.float32)
    proj_w = np.asarray(proj_w, dtype=np.float32)
    proj_b = np.asarray(proj_b, dtype=np.float32)

    scale = 1.0 / np.sqrt(C).astype(np.float32)
    Wq = qkv_w[0:C] * gn_w[None, :] * scale
    bq_eff = (qkv_w[0:C] @ gn_b + qkv_b[0:C]) * scale
    Wk = qkv_w[C:2 * C] * gn_w[None, :]
    Wv = qkv_w[2 * C:3 * C] * gn_w[None, :]
    bv_eff = qkv_w[2 * C:3 * C] @ gn_b + qkv_b[2 * C:3 * C]
    bp_eff = proj_b + proj_w @ bv_eff

    wqk = np.concatenate([Wq.T, Wk.T], axis=1).astype(np.float32)  # [C, 2C]
    wv_h = np.ascontiguousarray(Wv.T, dtype=np.float32)
    wp_h = np.ascontiguousarray(proj_w.T, dtype=np.float32)

    cidx = np.arange(P)
    gm = np.zeros((P, 16), dtype=np.float32)
    gm[cidx, cidx // GSIZE] = 1.0 / GSIZE
    gt = np.zeros((16, P), dtype=np.float32)
    gt[cidx // GSIZE, cidx] = 1.0

    shared = {
        "onr": np.ones((P, P), dtype=np.float32),
        "wqk": wqk,
        "wv": wv_h,
        "wp": wp_h,
        "bq": bq_eff.reshape(C, 1).astype(np.float32),
        "bp": bp_eff.reshape(C, 1).astype(np.float32),
        "gm": gm,
        "gt": gt,
    }

    xf = x.reshape(B, C, N)
    xs_per_core = []
    for core in range(NCORES):
        b, half = core // 2, core % 2
        if half == 0:
            xc = xf[b]
        else:
            xc = np.concatenate([xf[b][:, NH:], xf[b][:, :NH]], axis=1)
        xs_per_core.append(np.ascontiguousarray(xc))
    return shared, xs_per_core


def run_sharded(inputs, trace=False, trace_kwargs=None):
    """Run the 8-core kernel. Returns (full_output, BassKernelResults)."""
    from concourse.bass_utils import run_bass_kernel_spmd

    nc = _get_prog()
    shared, xs_per_core = _host_prep(**inputs)
    in_maps = [{**shared, "x": xs_per_core[c]} for c in range(NCORES)]
    kw = {}
    if trace:
        kw["trace"] = True
        if trace_kwargs:
            kw["trace_kwargs"] = trace_kwargs
    res = run_bass_kernel_spmd(nc, in_maps, list(range(NCORES)), **kw)

    out = np.empty((B, C, N), dtype=np.float32)
    for core in range(NCORES):
        b, half = core // 2, core % 2
        yc = res.results[core]["y"]
        out[b][:, half * NH:(half + 1) * NH] = yc
    return out.reshape(B, C, HH, WW), res


def kernel(**inputs):
    out, _ = run_sharded(inputs)
    return out
